# revision 1
# baseline (speedup 1.0000x reference)
"""Max-Feature-Map (pairwise max over adjacent channels) on 8 TRN2 cores.

Input  x: (32, 128, 112, 112) f32  ->  Output: (32, 64, 112, 112) f32
out[b, k] = max(x[b, 2k], x[b, 2k+1])   elementwise over the 112x112 plane.

Sharding: batch dim across the 8 cores (4 batches each, contiguous slice).
Per-core layout: the core's (4, 128, 112, 112) slice viewed as
(256 pairs, 2, 12544): pair p = channels (2k, 2k+1) of one batch, each a
contiguous 12544-float plane. Partition = pair, so the DVE max is a plain
free-dim tensor_tensor and every DMA is contiguous in DRAM.
"""

import numpy as np

import concourse.bass as bass
import concourse.mybir as mybir
import concourse.tile as tile
from concourse import bacc
from concourse.bass_utils import run_bass_kernel_spmd

N_CORES = 8
B, C, H, W = 32, 128, 112, 112
PLANE = H * W  # 12544
PAIRS = (B // N_CORES) * (C // 2)  # 256 channel-pairs per core
P = 128  # SBUF partitions
F = 6272  # free-dim chunk of the plane (6.4 MB loads / 3.2 MB stores)
N_CHUNKS = PLANE // F
IN_BUFS = 2
OUT_BUFS = 3


def _build_nc() -> bass.Bass:
    nc = bacc.Bacc()
    xin = nc.dram_tensor("x", [PAIRS, 2, PLANE], mybir.dt.float32, kind="ExternalInput")
    out = nc.dram_tensor("out", [PAIRS, PLANE], mybir.dt.float32, kind="ExternalOutput")
    with tile.TileContext(nc) as tc:
        with (
            tc.tile_pool(name="pin", bufs=IN_BUFS) as pin,
            tc.tile_pool(name="pout", bufs=OUT_BUFS) as pout,
        ):
            for pb in range(PAIRS // P):
                for j in range(N_CHUNKS):
                    t = pin.tile([P, 2, F], mybir.dt.float32)
                    nc.sync.dma_start(
                        t[:], xin[pb * P : (pb + 1) * P, :, j * F : (j + 1) * F]
                    )
                    o = pout.tile([P, F], mybir.dt.float32)
                    nc.vector.tensor_max(o[:], t[:, 0, :], t[:, 1, :])
                    nc.scalar.dma_start(
                        out[pb * P : (pb + 1) * P, j * F : (j + 1) * F], o[:]
                    )
    nc.finalize()
    return nc


def kernel(x):
    x = np.ascontiguousarray(np.asarray(x, dtype=np.float32))
    assert x.shape == (B, C, H, W)
    nc = _build_nc()
    per_core = x.reshape(N_CORES, PAIRS, 2, PLANE)
    in_maps = [{"x": per_core[c]} for c in range(N_CORES)]
    res = run_bass_kernel_spmd(nc, in_maps, core_ids=list(range(N_CORES)))
    full = np.stack([res.results[c]["out"] for c in range(N_CORES)])
    return full.reshape(B, C // 2, H, W)

